# revision 23
# baseline (speedup 1.0000x reference)
"""Trainium2 Bass kernel for nn_Abstraction (sparse_attention).

Reference computation (per batch element, N=4096, D=512, A=64):
    c      = l2_normalize(data, axis=-1)
    sim    = tril(c @ c.T)                      # [N, N] never materialized
    pooled = sim.reshape(N, N//A, A).mean(-2)   # [N, A]
    out    = concat([data, pooled @ W_abs], -1) @ W_merge

Identity: pooled[n, a] = (1/64) * r_n * x_n . ( sum_{g: g*A+a <= n} r_g x_g )
where x = raw rows, r = 1/||x||.  Per 128-row tile t (blocks 2t, 2t+1):
    w[n, a] = masked-sum over in-tile gram cols + x_n . (PSo_t[a] + baseT[a])
    pooled  = w * (r_n / 64)
with PSo_t = cumulative column-group sums of the own half (chain over tiles)
and baseT = column-group sums of the prefix half (zeros on lower cores).

Cost-model-driven choices:
  - all I/O fp16 (host casts); W_merge lower half folded into wcomb on host
  - C^T built by DMA-XBAR transpose straight from DRAM (no PE transposes)
  - sim uses raw-gram with the r-scale folded into the moving operand XcS,
    so no separate normalize pass over the data
  - merge = 4 data chunks + 1 pooled chunk into one PSUM bank per tile
  - base contribution via 4 extra matmuls in the bs accumulation group,
    decoupling the serial chain from the prefix half
  - elementwise work spread across DVE (4x-mode TSP), Pool, ACT
"""

import os
import sys

sys.path.insert(0, "/opt/trn_rl_repo")

import numpy as np

import concourse.bass as bass
import concourse.mybir as mybir
import concourse.tile as tile
from concourse import bacc
from concourse.bass_utils import run_bass_kernel_spmd
from concourse.masks import make_identity

F32 = mybir.dt.float32
F16 = mybir.dt.float16

B, N, D = 4, 4096, 512
A = 64            # abstraction (pool block) size
HALF = N // 2     # rows per core
NT = HALF // 128  # 128-row tiles per core (16)
KC = D // 128     # contraction chunks (4)
EPS = 1e-12

WARMUP = int(os.environ.get("K_WARMUP", "0"))
PE_POOLT_PAIRS = int(os.environ.get("K_PE_POOLT", "3"))  # early pairs via PE


def _build_nc():
    nc = bacc.Bacc(None, dynamic_dma_scratch_size=96 * 1024)

    xd = nc.dram_tensor("xd", [HALF, D], F16, kind="ExternalInput")
    xt = nc.dram_tensor("xt", [D, HALF], F16, kind="ExternalInput")
    xp = nc.dram_tensor("xp", [HALF, D], F16, kind="ExternalInput")
    wm = nc.dram_tensor("wm", [128, KC * D], F16, kind="ExternalInput")
    wc = nc.dram_tensor("wc", [2 * A, D], F16, kind="ExternalInput")
    out = nc.dram_tensor("out", [HALF, D], F16, kind="ExternalOutput")

    with tile.TileContext(nc) as tc:
        with (
            tc.tile_pool(name="persist", bufs=1) as pp,
            tc.tile_pool(name="ldx", bufs=2) as lxp,
            tc.tile_pool(name="ldp", bufs=2) as lpp,
            tc.tile_pool(name="work", bufs=3) as wp,
            tc.tile_pool(name="ostage", bufs=4) as osp,
            tc.tile_pool(name="psm", bufs=4, space="PSUM") as psm,
            tc.tile_pool(name="pssb", bufs=2, space="PSUM") as pssb,
            tc.tile_pool(name="psbt", bufs=1, space="PSUM") as psbt,
            tc.tile_pool(name="psmisc", bufs=1, space="PSUM") as psmisc,
            nc.allow_low_precision("fp16 matmul operands by design"),
        ):
            # ---------------- constants ----------------
            ident = pp.tile([128, 128], F16, tag="ident")
            make_identity(nc, ident)

            # mask2b[p, g, a] = 1.0 if 64g + a <= p else 0
            mask2b = pp.tile([128, 2, A], F32, tag="mask2b")
            nc.gpsimd.memset(mask2b, 1.0)
            nc.gpsimd.affine_select(
                out=mask2b, in_=mask2b,
                compare_op=mybir.AluOpType.is_ge, fill=0.0,
                base=0, pattern=[[-64, 2], [-1, A]], channel_multiplier=1,
            )

            # sel[p, a] = 1.0 if p % 64 == a else 0  (two stacked identities)
            sel = pp.tile([128, A], F16, tag="sel")
            nc.gpsimd.memset(sel, 0.0)
            for hh in range(2):
                nc.gpsimd.affine_select(
                    out=sel[hh * 64:(hh + 1) * 64, :],
                    in_=sel[hh * 64:(hh + 1) * 64, :],
                    compare_op=mybir.AluOpType.not_equal, fill=1.0,
                    base=0, pattern=[[-1, A]], channel_multiplier=1,
                )

            eps_sb = pp.tile([128, 1], F32, tag="eps")
            nc.vector.memset(eps_sb, EPS)

            # ---------------- persistent state ----------------
            Xc = pp.tile([128, KC, HALF], F16, tag="Xc")     # x^T
            XcS = pp.tile([128, KC, HALF], F16, tag="XcS")   # (r*x)^T
            PS = pp.tile([128, KC, NT, A], F16, tag="PS")    # own prefix sums
            baseT16 = pp.tile([128, KC, A], F16, tag="baseT16")
            wm16 = pp.tile([128, KC, D], F16, tag="wm16")
            wcomb = pp.tile([2 * A, D], F16, tag="wcomb")
            rrep = pp.tile([128, NT, 128], F16, tag="rrep")
            rts = pp.tile([1, NT, 128], F16, tag="rts")  # r bcast over parts
            ps_nat = pp.tile([128, NT, A], F16, tag="ps_nat")
            pooledT = pp.tile([128, NT // 2, 128], F16, tag="pooledT")

            sq = pp.tile([128, NT], F32, tag="sq")
            norm = pp.tile([128, NT], F32, tag="norm")
            r_ = pp.tile([128, NT], F32, tag="r")
            r16 = pp.tile([128, NT], F16, tag="r16")
            s64 = pp.tile([128, NT], F32, tag="s64")
            sqp = pp.tile([128, NT], F32, tag="sqp")
            normp = pp.tile([128, NT], F32, tag="normp")
            rp = pp.tile([128, NT], F32, tag="rp")

            # ---------------- PE warmup (p-state ramp) ----------------
            for i in range(WARMUP):
                warm = psmisc.tile([128, 128], F16, tag="misc",
                                   name=f"warm{i}")
                nc.tensor.transpose(warm, ident, ident)

            # ---------------- loads ----------------
            # weights + own row tiles on SP
            xrow = {}
            for g in range(4):
                t0 = lxp.tile([128, 4, D], F16, tag="xrow", name=f"xrow{g}")
                nc.sync.dma_start(
                    out=t0,
                    in_=xd[g * 512:(g + 1) * 512, :].rearrange(
                        "(t p) d -> p t d", p=128),
                )
                xrow[g] = t0
                if g == 0:
                    nc.sync.dma_start(out=wm16, in_=wm[:, :].rearrange(
                        "p (k o) -> p k o", k=KC))
                    nc.sync.dma_start(out=wcomb, in_=wc[:, :])

            # prefix row tiles: spread across Pool(SWDGE)+ACT so they land early
            xprow = {}
            for g in range(4):
                t1 = lpp.tile([128, 4, D], F16, tag="xprow", name=f"xprow{g}")
                eng = nc.gpsimd if g % 2 == 0 else nc.scalar
                eng.dma_start(
                    out=t1,
                    in_=xp[g * 512:(g + 1) * 512, :].rearrange(
                        "(t p) d -> p t d", p=128),
                )
                xprow[g] = t1

            # transposed own half via DMA-XBAR, quarter x chunk granularity
            for q in range(4):
                for k in range(KC):
                    nc.scalar.dma_start_transpose(
                        out=Xc[:, k, q * 512:(q + 1) * 512],
                        in_=xd[q * 512:(q + 1) * 512,
                               k * 128:(k + 1) * 128],
                    )

            # ---------------- per-tile norm pipelines ----------------
            def own_norm(t):
                g, i = divmod(t, 4)
                scr = wp.tile([128, D], F16, tag="scr", name=f"scr{t}")
                # sq accumulation via DVE 4x-mode tensor-scalar
                nc.vector.scalar_tensor_tensor(
                    out=scr, in0=xrow[g][:, i, :], scalar=1.0,
                    in1=xrow[g][:, i, :],
                    op0=mybir.AluOpType.mult, op1=mybir.AluOpType.mult,
                    accum_out=sq[:, t:t + 1],
                )
                nc.scalar.activation(
                    out=norm[:, t:t + 1], in_=sq[:, t:t + 1],
                    func=mybir.ActivationFunctionType.Sqrt, bias=eps_sb,
                )
                nc.vector.reciprocal(out=r_[:, t:t + 1], in_=norm[:, t:t + 1])
                nc.vector.tensor_scalar_mul(r16[:, t:t + 1], r_[:, t:t + 1],
                                            0.125)
                nc.vector.tensor_scalar_mul(
                    s64[:, t:t + 1], r_[:, t:t + 1], 1.0 / 64.0)

            def prefix_norm(t):
                g, i = divmod(t, 4)
                scr = wp.tile([128, D], F16, tag="pscr", name=f"pscr{t}")
                eng = nc.gpsimd if t % 2 == 0 else nc.scalar
                if t % 2 == 0:
                    eng.scalar_tensor_tensor(
                        out=scr, in0=xprow[g][:, i, :], scalar=1.0,
                        in1=xprow[g][:, i, :],
                        op0=mybir.AluOpType.mult, op1=mybir.AluOpType.mult,
                        accum_out=sqp[:, t:t + 1],
                    )
                else:
                    nc.scalar.activation(
                        out=scr, in_=xprow[g][:, i, :],
                        func=mybir.ActivationFunctionType.Square,
                        accum_out=sqp[:, t:t + 1],
                    )
                nc.scalar.activation(
                    out=normp[:, t:t + 1], in_=sqp[:, t:t + 1],
                    func=mybir.ActivationFunctionType.Sqrt, bias=eps_sb,
                )
                nc.vector.reciprocal(out=rp[:, t:t + 1], in_=normp[:, t:t + 1])

            bT = psbt.tile([128, KC, 128], F32, tag="bT", name="bT")
            xpS_all = pp.tile([128, NT, D], F16, tag="xpS")

            def prefix_base(t):
                # xpS = xp * rp  (per-partition TSP, 4x on DVE)
                g, i = divmod(t, 4)
                nc.vector.tensor_scalar_mul(xpS_all[:, t, :],
                                            xprow[g][:, i, :],
                                            rp[:, t:t + 1])

            def own_scaled(t):
                # rrep[:, t, :] = broadcast of r16 col t across partitions,
                # then XcS tile = Xc tile * rrep  (fp16 TT, 2x mode)
                nc.vector.tensor_tensor(
                    XcS[:, :, t * 128:(t + 1) * 128],
                    Xc[:, :, t * 128:(t + 1) * 128],
                    rrep[:, t, :][:, None, :].to_broadcast((128, KC, 128)),
                    mybir.AluOpType.mult,
                )
                if t < NT - 1:
                    # presum for chain: PS[t+1] = XcS_h0 + XcS_h1 (of tile t)
                    nc.vector.tensor_tensor(
                        PS[:, :, t + 1, :],
                        XcS[:, :, t * 128:t * 128 + 64],
                        XcS[:, :, t * 128 + 64:(t + 1) * 128],
                        mybir.AluOpType.add,
                    )

            def rt_group(g):
                # transpose r16[:, 4g:4g+4] -> [4, 128] then broadcast rows
                rtp = psmisc.tile([128, 128], F16, tag="misc",
                                  name=f"rtp{g}")
                nc.tensor.transpose(rtp[0:4, :], r16[:, 4 * g:4 * g + 4],
                                    ident)
                for i in range(4):
                    nc.gpsimd.partition_broadcast(
                        rrep[:, 4 * g + i, :], rtp[i:i + 1, :])

            # ---- emission: norms + base + scaled transposes ----
            for g in range(4):
                for t in range(4 * g, 4 * g + 4):
                    own_norm(t)
                    prefix_norm(t)
                    prefix_base(t)
                rt_group(g)
                for t in range(4 * g, 4 * g + 4):
                    own_scaled(t)

            # baseT accumulation: k-outer so each zero-region group is
            # opened and closed sequentially within the single bT bank
            for k in range(KC):
                for t in range(NT):
                    nc.tensor.matmul(
                        bT[:, k, 0:A],
                        xpS_all[:, t, k * 128:(k + 1) * 128], sel,
                        start=(t == 0), stop=(t == NT - 1),
                    )
            nc.vector.tensor_copy(out=baseT16, in_=bT[:, :, 0:A])
            # own-only chain start: PS[0] = 0
            nc.vector.memset(PS[:, :, 0, :], 0.0)
            # PSb[0] = baseT16 (PS[0] = 0)
            nc.gpsimd.tensor_tensor(
                PSb[:, :, 0, :], PS[:, :, 0, :], baseT16,
                mybir.AluOpType.add)
            for t in range(1, NT):
                nc.vector.tensor_tensor(
                    PS[:, :, t, :], PS[:, :, t, :], PS[:, :, t - 1, :],
                    mybir.AluOpType.add,
                )

            # ---------------- sim + bs + combine per tile ----------------
            sb_tiles = {}

            def sim_bs(t):
                SB = pssb.tile([128, 192], F32, tag="SB", name=f"SB{t}")
                sb_tiles[t] = SB
                for k in range(KC):
                    nc.tensor.matmul(
                        SB[:, 0:128],
                        Xc[:, k, t * 128:(t + 1) * 128],
                        XcS[:, k, t * 128:(t + 1) * 128],
                        start=(k == 0), stop=(k == KC - 1),
                    )
                for k in range(KC):
                    nc.tensor.matmul(
                        SB[:, 128:192],
                        Xc[:, k, t * 128:(t + 1) * 128],
                        PS[:, k, t, :],
                        start=(k == 0), stop=False,
                    )
                for k in range(KC):
                    nc.tensor.matmul(
                        SB[:, 128:192],
                        Xc[:, k, t * 128:(t + 1) * 128],
                        baseT16[:, k, :],
                        start=False, stop=(k == KC - 1),
                    )

            def combine(t):
                SB = sb_tiles[t]
                u = wp.tile([128, 2, A], F32, tag="u", name=f"u{t}")
                nc.vector.tensor_tensor(
                    u, SB[:, 0:128].rearrange("p (g a) -> p g a", g=2),
                    mask2b, mybir.AluOpType.mult,
                )
                v = wp.tile([128, A], F32, tag="v", name=f"v{t}")
                nc.gpsimd.tensor_tensor(
                    v, u[:, 0, :], u[:, 1, :], mybir.AluOpType.add)
                w = wp.tile([128, A], F32, tag="w", name=f"w{t}")
                nc.vector.tensor_tensor(
                    w, v, SB[:, 128:192], mybir.AluOpType.add)
                nc.gpsimd.tensor_scalar_mul(
                    ps_nat[:, t, :], w, s64[:, t:t + 1])

            def poolt_pair(g):
                # pooledT[:, g, :] = transpose of ps_nat[:, 2g:2g+2, :]
                if g < PE_POOLT_PAIRS:
                    ptp = psmisc.tile([128, 128], F16, tag="misc",
                                      name=f"ptp{g}")
                    nc.tensor.transpose(
                        ptp, ps_nat[:, 2 * g:2 * g + 2, :], ident)
                    nc.vector.tensor_copy(out=pooledT[:, g, :], in_=ptp)
                else:
                    nc.scalar.dma_start_transpose(
                        out=pooledT[:, g, :],
                        in_=ps_nat[:, 2 * g:2 * g + 2, :],
                    )

            def merge(t):
                M = psm.tile([128, D], F32, tag="M", name=f"M{t}")
                for k in range(KC):
                    nc.tensor.matmul(
                        M, Xc[:, k, t * 128:(t + 1) * 128], wm16[:, k, :],
                        start=(k == 0), stop=False,
                    )
                j = t % 2
                nc.tensor.matmul(
                    M, pooledT[j * 64:(j + 1) * 64, t // 2, :],
                    wcomb[j * 64:(j + 1) * 64, :],
                    start=False, stop=True,
                )
                osb = osp.tile([128, D], F16, tag="osb", name=f"osb{t}")
                nc.gpsimd.tensor_copy(out=osb, in_=M)
                nc.sync.dma_start(out=out[t * 128:(t + 1) * 128, :], in_=osb)

            # pipeline: sims/bs lead, merges trail by 2 tiles
            LAG = 3
            for t in range(NT):
                sim_bs(t)
                combine(t)
                if t % 2 == 1:
                    poolt_pair(t // 2)
                if t >= LAG:
                    merge(t - LAG)
            for t in range(NT - LAG, NT):
                merge(t)

    nc.finalize()
    return nc


_NC_CACHE = None


def _get_nc():
    global _NC_CACHE
    if _NC_CACHE is None:
        _NC_CACHE = _build_nc()
    return _NC_CACHE


def host_inputs(data, W_abs, W_merge):
    """Per-core input dicts (host-side shard + dtype staging)."""
    data = np.asarray(data, dtype=np.float32)
    W_abs = np.asarray(W_abs, dtype=np.float32)
    W_merge = np.asarray(W_merge, dtype=np.float32)
    d16 = data.astype(np.float16)
    wm_host = np.ascontiguousarray(
        W_merge[:D].astype(np.float16).reshape(KC, 128, D)
        .transpose(1, 0, 2).reshape(128, KC * D))
    wc1 = (W_abs @ W_merge[D:]).astype(np.float16)
    wc_host = np.ascontiguousarray(np.concatenate([wc1, wc1], axis=0))
    zeros_half = np.zeros((HALF, D), np.float16)
    in_maps = []
    for core in range(8):
        b, h = divmod(core, 2)
        half = d16[b, h * HALF:(h + 1) * HALF]
        in_maps.append({
            "xd": np.ascontiguousarray(half),
            "xt": np.ascontiguousarray(half.T),
            "xp": np.ascontiguousarray(d16[b, 0:HALF]) if h == 1
            else zeros_half,
            "wm": wm_host,
            "wc": wc_host,
        })
    return in_maps


_RUNNER = None


def _get_runner():
    """Build (once) a cached jitted SPMD executor for the 8-core kernel."""
    global _RUNNER
    if _RUNNER is not None:
        return _RUNNER

    import jax
    from jax.sharding import Mesh, PartitionSpec
    from jax.experimental.shard_map import shard_map

    import concourse.mybir as mybir
    from concourse import bass2jax

    bass2jax.install_neuronx_cc_hook()
    nc = _get_nc()
    n_cores = 8

    partition_name = (
        nc.partition_id_tensor.name if nc.partition_id_tensor else None
    )
    in_names, out_names, out_shapes, out_dtypes, zero_outs = [], [], [], [], []
    for alloc in nc.m.functions[0].allocations:
        if not isinstance(alloc, mybir.MemoryLocationSet):
            continue
        name = alloc.memorylocations[0].name
        if alloc.kind == "ExternalInput":
            if name != partition_name:
                in_names.append(name)
        elif alloc.kind == "ExternalOutput":
            shape = tuple(alloc.tensor_shape)
            dtype = mybir.dt.np(alloc.dtype)
            out_names.append(name)
            out_shapes.append(shape)
            out_dtypes.append(dtype)
            zero_outs.append(np.zeros(shape, dtype))
    n_params = len(in_names)
    out_avals = [
        jax.core.ShapedArray(s, d) for s, d in zip(out_shapes, out_dtypes)
    ]
    all_in_names = list(in_names) + list(out_names)
    if partition_name is not None:
        all_in_names.append(partition_name)
    donate = tuple(range(n_params, n_params + len(out_names)))

    def _body(*args):
        operands = list(args)
        if partition_name is not None:
            operands.append(bass2jax.partition_id_tensor())
        outs = bass2jax._bass_exec_p.bind(
            *operands,
            out_avals=tuple(out_avals),
            in_names=tuple(all_in_names),
            out_names=tuple(out_names),
            lowering_input_output_aliases=(),
            sim_require_finite=True,
            sim_require_nnan=True,
            nc=nc,
        )
        return tuple(outs)

    devices = jax.devices()[:n_cores]
    mesh = Mesh(np.asarray(devices), ("core",))
    in_specs = (PartitionSpec("core"),) * (n_params + len(out_names))
    out_specs = (PartitionSpec("core"),) * len(out_names)
    sharded = jax.jit(
        shard_map(
            _body, mesh=mesh, in_specs=in_specs, out_specs=out_specs,
            check_rep=False,
        ),
        donate_argnums=donate,
        keep_unused=True,
    )
    _RUNNER = (sharded, in_names, out_names, out_shapes, zero_outs, n_cores)
    return _RUNNER


def _run_fast(in_maps):
    sharded, in_names, out_names, out_shapes, zero_outs, n_cores = _get_runner()
    concat_in = [
        np.concatenate([in_maps[c][nm] for c in range(n_cores)], axis=0)
        for nm in in_names
    ]
    big_zeros = [
        np.zeros((n_cores * z.shape[0],) + z.shape[1:], z.dtype)
        for z in zero_outs
    ]
    out_arrs = sharded(*concat_in, *big_zeros)
    return [
        {
            nm: np.asarray(out_arrs[i]).reshape(
                (n_cores,) + out_shapes[i])[c]
            for i, nm in enumerate(out_names)
        }
        for c in range(n_cores)
    ]


def kernel(data, W_abs, W_merge, _trace=False):
    assert np.asarray(data).shape == (B, N, D)
    in_maps = host_inputs(data, W_abs, W_merge)

    if _trace:
        nc = _get_nc()
        res = run_bass_kernel_spmd(
            nc, in_maps, core_ids=list(range(8)), trace=True,
            stitch_traces=True,
        )
        results = res.results
    else:
        res = None
        results = _run_fast(in_maps)

    out = np.empty((B, N, D), np.float32)
    for core in range(8):
        b, h = divmod(core, 2)
        out[b, h * HALF:(h + 1) * HALF] = (
            results[core]["out"].astype(np.float32))
    if _trace:
        return out, res
    return out
